# revision 32
# baseline (speedup 1.0000x reference)
"""Trainium2 Bass kernel for nn_AnisotropicStack (ragged EMA tokenizer/detokenizer).

Self-contained: builds + compiles an 8-core SPMD Bass kernel (one batch row
per core), runs via bass_utils.run_bass_kernel_spmd, returns (output, new_state).

Algorithm per core (batch row):
  1. Host precomputes compaction indices (selected token positions), chunk
     probs, and expansion chunk-ids from the boolean mask (tiny int work).
  2. Device gathers the M=2048 selected x rows via indirect DMA into a
     (128 chunks x 16 steps) layout, runs the EMA scan:
        pass1: in-chunk scan (16 scalar_tensor_tensor steps, chunks on partitions)
        pass2: cross-chunk carry via log-space prefix + masked-exp matmul
        pass3: apply carry -> bf16 EMA table (SBUF-resident)
  3. Table rearranged to row-major partitions (SBUF->SBUF DMA); scatter-expand
     done as PE matmuls: out_tile = R^T @ table_window + residual, where
     R^T[i,l] = (ci[l] == window_base + i) is built on-device by is_equal
     against an iota column. Windows are static per l-tile (ci[l] ~ l/4 with
     bounded deviation, validated on host at runtime; rebuilt wider if needed).
  4. Row 8192 of the output carries new_state.
"""
import sys

for _p in ("/opt/trn_rl_repo",):
    if _p not in sys.path:
        sys.path.append(_p)

import numpy as np

B, L, D = 8, 8192, 1024
M = L // 4            # 2048 chunk slots
NCH, JW = 128, 16     # chunks on partitions, steps within chunk (NCH*JW == M)
NLT = L // 128        # 64 l-tiles for expansion
NTB = M // 128        # 16 table blocks of 128 rows
CIMARK = float(M)     # out-of-range marker for invalid positions

_CACHE = {}


def _window_starts(nblk, ranges=None):
    """Static table-block window start (in blocks) per l-tile (plus the
    new_state pseudo-tile at index NLT).

    ranges: optional per-tile (lo, hi) ci bounds aggregated over all cores;
    when given, windows are placed to cover them exactly."""
    ws = []
    for k in range(NLT + 1):
        if ranges is not None and k < len(ranges) and ranges[k] is not None:
            lo, hi = ranges[k]
            w = hi // 128 - (nblk - 1)          # rightmost block holds hi
            w = min(w, lo // 128)               # but still cover lo
            w = max(0, min(w, NTB - nblk))
        elif k >= NLT:
            w = NTB - nblk                      # new_state: last blocks
        else:
            w = max(0, min((32 * k - 62) // 128, NTB - nblk))
        ws.append(w)
    return ws


def _tile_ranges(ci_list, last_list):
    """Per-l-tile (lo, hi) of valid ci values across all cores, plus the
    new_state pseudo-tile (index NLT) covering each core's last row."""
    ranges = []
    for k in range(NLT):
        lo, hi = None, None
        for ci in ci_list:
            seg = ci[k * 128:(k + 1) * 128]
            v = seg[seg >= 0]
            if v.size:
                lo = int(v.min()) if lo is None else min(lo, int(v.min()))
                hi = int(v.max()) if hi is None else max(hi, int(v.max()))
        ranges.append(None if lo is None else (lo, hi))
    ranges.append((min(last_list), max(last_list)))
    return ranges


def _build(nblk, ranges=None):
    import concourse.bacc as bacc
    import concourse.mybir as mybir
    import concourse.tile as tile
    from concourse.bass import IndirectOffsetOnAxis

    f32 = mybir.dt.float32
    f16 = mybir.dt.float16
    bf16 = mybir.dt.bfloat16
    i32 = mybir.dt.int32
    op = mybir.AluOpType
    AF = mybir.ActivationFunctionType

    nc = bacc.Bacc("TRN2", target_bir_lowering=False)

    x_d = nc.dram_tensor("x", (L, D), f32, kind="ExternalInput")
    res_d = nc.dram_tensor("res", (L, D), f32, kind="ExternalInput")
    state_d = nc.dram_tensor("state", (1, D), f32, kind="ExternalInput")
    selidx_d = nc.dram_tensor("selidx", (NCH, JW), i32, kind="ExternalInput")
    cp_d = nc.dram_tensor("cp", (NCH, JW), f32, kind="ExternalInput")
    cif_d = nc.dram_tensor("cif", (1, L + 128), f16, kind="ExternalInput")
    out_d = nc.dram_tensor("out", (L + 1, D), f32, kind="ExternalOutput")

    # constants
    kk = np.arange(128)[:, None]
    mm = np.arange(128)[None, :]
    cst_np = np.concatenate([
        (kk <= mm).astype(np.float32),
        (kk < mm).astype(np.float32),
        np.where(kk < mm, 0.0, -1e5).astype(np.float32),
        np.arange(128, dtype=np.float32).reshape(128, 1)], axis=1)
    cst_d = nc.inline_tensor(cst_np, name="cst")
    ones_d = nc.inline_tensor(np.ones((1, 128), np.float16), name="ones")

    wstart = _window_starts(nblk, ranges)

    with tile.TileContext(nc) as tc:
        with tc.tile_pool(name="persist", bufs=1) as pp, \
             tc.tile_pool(name="resp", bufs=18) as resp, \
             tc.tile_pool(name="psum", bufs=1, space="PSUM") as psp, \
             tc.tile_pool(name="psum_scan", bufs=1, space="PSUM") as pss:

            # ---- small loads ----
            selidx_t = pp.tile([NCH, JW], i32)
            nc.sync.dma_start(out=selidx_t[:], in_=selidx_d[:])
            sm_t = pp.tile([128, 195], f32)
            cp_t = sm_t[:, 0:16]
            nc.sync.dma_start(out=cp_t, in_=cp_d[:])
            state_t = pp.tile([1, D], f32)
            nc.sync.dma_start(out=state_t[:], in_=state_d[:])
            cst_t = pp.tile([128, 385], f32)
            nc.sync.dma_start(out=cst_t[:], in_=cst_d[:])
            lti_t = cst_t[:, 0:128]
            lts_t = cst_t[:, 128:256]
            mb_t = cst_t[:, 256:384]
            iota_t = cst_t[:, 384:385]
            cif_t = pp.tile([1, L + 128], f16)
            nc.sync.dma_start(out=cif_t[:], in_=cif_d[:])
            ones_t = pp.tile([1, 128], f16)
            nc.sync.dma_start(out=ones_t[:], in_=ones_d[:])

            # table, row-major partitions: row t -> partition t%128, block t//128
            TB = pp.tile([128, NTB * D], bf16)

            with tc.tile_pool(name="scanp", bufs=1) as sp:
                # ---- gather selected x rows: CH[c, j*D:(j+1)*D] = x[sel[c*16+j]]
                CH = sp.tile([NCH, JW * D], bf16)
                for j in range(JW):
                    nc.gpsimd.indirect_dma_start(
                        out=CH[:, j * D:(j + 1) * D],
                        out_offset=None,
                        in_=x_d[:],
                        in_offset=IndirectOffsetOnAxis(ap=selidx_t[:, j:j + 1],
                                                       axis=0),
                    )

                # ---- decay + cumprod ----
                dec_t = sm_t[:, 16:32]
                nc.vector.tensor_scalar(out=dec_t, in0=cp_t, scalar1=-1.0,
                                        scalar2=1.0, op0=op.mult, op1=op.add)
                z16 = sm_t[:, 32:48]
                nc.vector.memset(z16, 0.0)
                P_t = sm_t[:, 48:64]
                nc.vector.tensor_tensor_scan(out=P_t, data0=dec_t,
                                             data1=z16, initial=1.0,
                                             op0=op.mult, op1=op.add)

                # ---- cross-chunk carry weights W (only needs cp; emitted early
                # so the scalar engine's Ln/Exp precede its res-DMA stream)
                A_t = sm_t[:, 64:65]
                nc.vector.tensor_scalar(out=A_t, in0=P_t[:, JW - 1:JW],
                                        scalar1=1e-38, scalar2=None, op0=op.max)
                Lc_t = sm_t[:, 65:66]
                nc.scalar.activation(out=Lc_t, in_=A_t, func=AF.Ln)
                nc.vector.tensor_scalar(out=Lc_t, in0=Lc_t, scalar1=-87.0,
                                        scalar2=None, op0=op.max)
                S_ps = pss.tile([128, 1], f32, name="S_ps", tag="small_ps", bufs=2)
                nc.tensor.matmul(out=S_ps[:], lhsT=lti_t, rhs=Lc_t,
                                 start=True, stop=True)
                S_t = sm_t[:, 66:67]
                nc.vector.tensor_copy(out=S_t, in_=S_ps[:])
                S2b_ps = pss.tile([128, 128], f32, name="S2b_ps", tag="small_ps", bufs=2)
                nc.tensor.matmul(out=S2b_ps[:],
                                 lhsT=Lc_t.to_broadcast([128, 128]),
                                 rhs=lts_t, start=True, stop=True)
                W_t = sm_t[:, 67:195]  # f32 workspace for subtract/min
                nc.vector.tensor_scalar(out=W_t, in0=S2b_ps[:],
                                        scalar1=S_t, scalar2=None,
                                        op0=op.subtract)
                nc.vector.tensor_tensor(out=W_t, in0=W_t, in1=mb_t,
                                        op=op.min)
                W_bf = pp.tile([128, 128], bf16)
                nc.scalar.activation(out=W_bf[:], in_=W_t, func=AF.Exp)

                # ---- prefetch first residual tiles (sync queue; the rest
                # are emitted per-tile in the expansion loop so the sync
                # queue's slot waits follow the adds that free them) ----
                res_tiles = []
                for t in range(18):
                    r = resp.tile([128, D], f32, name="r", tag="r")
                    nc.sync.dma_start(out=r[:],
                                      in_=res_d[t * 128:(t + 1) * 128, :])
                    res_tiles.append(r)

                # ---- b scale (in place): CH_j = cp_j * CH_j ----
                for j in range(JW):
                    nc.vector.tensor_scalar(out=CH[:, j * D:(j + 1) * D],
                                            in0=CH[:, j * D:(j + 1) * D],
                                            scalar1=cp_t[:, j:j + 1],
                                            scalar2=None, op0=op.mult)
                # fold initial state into b[0] (chunk 0): b0 += dec0 * state
                nc.vector.scalar_tensor_tensor(out=CH[0:1, 0:D], in0=state_t[:],
                                               scalar=dec_t[0:1, 0:1],
                                               in1=CH[0:1, 0:D],
                                               op0=op.mult, op1=op.add)

                # ---- pass1: in-chunk scan, in place ----
                for j in range(1, JW):
                    nc.vector.scalar_tensor_tensor(
                        out=CH[:, j * D:(j + 1) * D],
                        in0=CH[:, (j - 1) * D:j * D],
                        scalar=dec_t[:, j:j + 1],
                        in1=CH[:, j * D:(j + 1) * D],
                        op0=op.mult, op1=op.add)

                # ---- pass2: carry[c] = sum_{c'<c} W[c',c] * Hend[c'] ----
                carry_ps = pss.tile([128, D], f32, name="carry_ps",
                                    tag="carry_ps")
                hend = CH[:, (JW - 1) * D:JW * D]
                for h in range(2):
                    nc.tensor.matmul(out=carry_ps[:, h * 512:(h + 1) * 512],
                                     lhsT=W_bf[:],
                                     rhs=hend[:, h * 512:(h + 1) * 512],
                                     start=True, stop=True)

                # ---- pass3: E = H + P_j * carry -> bf16 ----
                E_bf = sp.tile([NCH, JW * D], bf16)
                for j in range(JW):
                    nc.vector.scalar_tensor_tensor(
                        out=E_bf[:, j * D:(j + 1) * D],
                        in0=carry_ps[:],
                        scalar=P_t[:, j:j + 1],
                        in1=CH[:, j * D:(j + 1) * D],
                        op0=op.mult, op1=op.add)

                # ---- rearrange (c,j) chunk layout -> row-major blocks ----
                # TB[p, w*D + d] = E(t = 128w + p) = E_bf[8w + p//16, (p%16)*D + d]
                for w in range(NTB):
                    dma_eng = nc.gpsimd if w % 2 == 0 else nc.sync
                    dma_eng.dma_start(
                        out=TB[:, w * D:(w + 1) * D],
                        in_=E_bf[8 * w:8 * w + 8, :])

            # ---- expansion: out_tile = R^T @ table_window + residual ----
            rtp_cm = tc.tile_pool(name="rtp", bufs=4)
            rtp = rtp_cm.__enter__()
            outp_cm = tc.tile_pool(name="outp", bufs=4)
            outp = outp_cm.__enter__()
            for k in range(NLT + 1):
                # ci values for this tile broadcast down all 128 partitions
                cb_ps = pss.tile([128, 128], f32, name="cb_ps", tag="small_ps",
                                 bufs=2)
                nc.tensor.matmul(out=cb_ps[:], lhsT=ones_t[:],
                                 rhs=cif_t[0:1, k * 128:(k + 1) * 128],
                                 start=True, stop=True)
                exp_ps = psp.tile([128, D], f32, name="exp_ps", tag="exp_ps",
                                  bufs=2)
                for bi in range(nblk):
                    w = wstart[k] + bi
                    # R^T[i, l] = (ci[l] - i == 128*w)
                    rt = rtp.tile([128, 128], bf16, name="rt", tag="rt",
                                  bufs=4)
                    nc.vector.tensor_scalar(out=rt[:], in0=cb_ps[:],
                                            scalar1=iota_t,
                                            scalar2=float(128 * w),
                                            op0=op.subtract, op1=op.is_equal)
                    for h in range(2):
                        nc.tensor.matmul(
                            out=exp_ps[:, h * 512:(h + 1) * 512],
                            lhsT=rt[:],
                            rhs=TB[:, w * D + h * 512:w * D + (h + 1) * 512],
                            start=(bi == 0), stop=(bi == nblk - 1))
                if k < NLT:
                    # scalar engine drains PSUM; vector adds at SBUF 2x rate
                    g = outp.tile([128, D], f32, name="g", tag="g")
                    nc.scalar.activation(out=g[:], in_=exp_ps[:], func=AF.Copy)
                    o = outp.tile([128, D], f32, name="o", tag="o")
                    nc.vector.tensor_tensor(out=o[:], in0=res_tiles[k][:],
                                            in1=g[:], op=op.add)
                    nc.sync.dma_start(out=out_d[k * 128:(k + 1) * 128, :],
                                      in_=o[:])
                    t2 = k + 18
                    if t2 < NLT:
                        r = resp.tile([128, D], f32, name="r", tag="r")
                        nc.sync.dma_start(out=r[:],
                                          in_=res_d[t2 * 128:(t2 + 1) * 128, :])
                        res_tiles.append(r)
                else:
                    # tile NLT: every partition holds table[last_idx];
                    # row 0 is new_state
                    ns = outp.tile([1, D], f32, name="ns", tag="ns")
                    nc.vector.tensor_copy(out=ns[:], in_=exp_ps[0:1, :])
                    nc.sync.dma_start(out=out_d[L:L + 1, :], in_=ns[:])
            outp_cm.__exit__(None, None, None)
            rtp_cm.__exit__(None, None, None)

    nc.compile()
    return nc


def _host_prep(prob_row, mask_row):
    """Per-row index/prob prep. Returns (aux input dict, counts, max window excess)."""
    mask = mask_row.astype(bool)
    counts = int(mask.sum())
    sel = np.argsort(~mask, kind="stable")[:M].astype(np.int32)
    valid = (np.arange(M) < counts)
    cp = (prob_row[sel] * valid).astype(np.float32)
    chunk_idx = np.cumsum(mask.astype(np.int64)) - 1
    ci = np.clip(chunk_idx, 0, M - 1)
    last_idx = (counts - 1) if counts > 0 else 0
    cif = np.concatenate([
        np.where(chunk_idx >= 0, ci, M),
        np.full(128, last_idx, np.int64)]).astype(np.float16)
    return {
        "selidx": np.ascontiguousarray(sel.reshape(NCH, JW)),
        "cp": np.ascontiguousarray(cp.reshape(NCH, JW)),
        "cif": np.ascontiguousarray(cif.reshape(1, L + 128)),
    }, counts, ci, last_idx


def _needed_nblk(ci_list, last_list):
    """Smallest nblk whose data-placed static windows cover every tile."""
    ranges = _tile_ranges(ci_list, last_list)
    for nblk in range(2, NTB + 1):
        ws = _window_starts(nblk, ranges)
        ok = True
        for k in range(NLT + 1):
            if ranges[k] is None:
                continue
            lo, hi = ranges[k]
            if lo < 128 * ws[k] or hi >= 128 * (ws[k] + nblk):
                ok = False
                break
        if ok:
            return nblk, ranges
    return NTB, ranges


def kernel(x, residual, prob, token_mask, state):
    from concourse import bass_utils

    auxs, countss, cis, lasts = [], [], [], []
    for b in range(B):
        aux, counts, ci, last_idx = _host_prep(np.asarray(prob[b]),
                                               np.asarray(token_mask[b]))
        auxs.append(aux)
        countss.append(counts)
        lasts.append(last_idx)
        cis.append(np.where(np.asarray(token_mask[b]).astype(bool).cumsum() > 0,
                            ci, -1))
    nblk, ranges = _needed_nblk(cis, lasts)

    key = (nblk, tuple(r if r is None else tuple(r) for r in ranges))
    if key not in _CACHE:
        _CACHE[key] = _build(nblk, ranges)
    nc = _CACHE[key]

    in_maps = []
    for b in range(B):
        in_maps.append({
            "x": np.ascontiguousarray(x[b], dtype=np.float32),
            "res": np.ascontiguousarray(residual[b], dtype=np.float32),
            "state": np.ascontiguousarray(state[b],
                                          dtype=np.float32).reshape(1, D),
            **auxs[b],
        })

    res = bass_utils.run_bass_kernel_spmd(nc, in_maps, core_ids=list(range(B)))
    output = np.stack([res.results[b]["out"][:L] for b in range(B)])
    new_state = np.stack([res.results[b]["out"][L] for b in range(B)])
    for b in range(B):
        if countss[b] == 0:
            # no selected tokens: new_state passes the input state through
            new_state[b] = np.asarray(state[b])
    return output, new_state


# revision 33
# speedup vs baseline: 1.3893x; 1.3893x over previous
"""Trainium2 Bass kernel for nn_AnisotropicStack (ragged EMA tokenizer/detokenizer).

Self-contained: builds + compiles an 8-core SPMD Bass kernel (one batch row
per core), runs via bass_utils.run_bass_kernel_spmd, returns (output, new_state).

Algorithm per core (batch row):
  1. Host precomputes compaction indices (selected token positions), chunk
     probs, and expansion chunk-ids from the boolean mask (tiny int work).
  2. Device gathers the M=2048 selected x rows via indirect DMA into a
     (128 chunks x 16 steps) layout, runs the EMA scan:
        pass1: in-chunk scan (16 scalar_tensor_tensor steps, chunks on partitions)
        pass2: cross-chunk carry via log-space prefix + masked-exp matmul
        pass3: apply carry -> bf16 EMA table (SBUF-resident)
  3. Table rearranged to row-major partitions (SBUF->SBUF DMA); scatter-expand
     done as PE matmuls: out_tile = R^T @ table_window + residual, where
     R^T[i,l] = (ci[l] == window_base + i) is built on-device by is_equal
     against an iota column. Windows are static per l-tile (ci[l] ~ l/4 with
     bounded deviation, validated on host at runtime; rebuilt wider if needed).
  4. Row 8192 of the output carries new_state.
"""
import sys

for _p in ("/opt/trn_rl_repo",):
    if _p not in sys.path:
        sys.path.append(_p)

import numpy as np

B, L, D = 8, 8192, 1024
M = L // 4            # 2048 chunk slots
NCH, JW = 128, 16     # chunks on partitions, steps within chunk (NCH*JW == M)
NLT = L // 128        # 64 l-tiles for expansion
NTB = M // 128        # 16 table blocks of 128 rows
CIMARK = float(M)     # out-of-range marker for invalid positions

_CACHE = {}


def _window_starts(nblk, ranges=None):
    """Static table-block window start (in blocks) per l-tile (plus the
    new_state pseudo-tile at index NLT).

    ranges: optional per-tile (lo, hi) ci bounds aggregated over all cores;
    when given, windows are placed to cover them exactly."""
    ws = []
    for k in range(NLT + 1):
        if ranges is not None and k < len(ranges) and ranges[k] is not None:
            lo, hi = ranges[k]
            w = hi // 128 - (nblk - 1)          # rightmost block holds hi
            w = min(w, lo // 128)               # but still cover lo
            w = max(0, min(w, NTB - nblk))
        elif k >= NLT:
            w = NTB - nblk                      # new_state: last blocks
        else:
            w = max(0, min((32 * k - 62) // 128, NTB - nblk))
        ws.append(w)
    return ws


def _tile_ranges(ci_list, last_list):
    """Per-l-tile (lo, hi) of valid ci values across all cores, plus the
    new_state pseudo-tile (index NLT) covering each core's last row."""
    ranges = []
    for k in range(NLT):
        lo, hi = None, None
        for ci in ci_list:
            seg = ci[k * 128:(k + 1) * 128]
            v = seg[seg >= 0]
            if v.size:
                lo = int(v.min()) if lo is None else min(lo, int(v.min()))
                hi = int(v.max()) if hi is None else max(hi, int(v.max()))
        ranges.append(None if lo is None else (lo, hi))
    ranges.append((min(last_list), max(last_list)))
    return ranges


def _build(nblk, ranges=None):
    import concourse.bacc as bacc
    import concourse.mybir as mybir
    import concourse.tile as tile
    from concourse.bass import IndirectOffsetOnAxis

    f32 = mybir.dt.float32
    f16 = mybir.dt.float16
    bf16 = mybir.dt.bfloat16
    i32 = mybir.dt.int32
    op = mybir.AluOpType
    AF = mybir.ActivationFunctionType

    nc = bacc.Bacc("TRN2", target_bir_lowering=False)

    x_d = nc.dram_tensor("x", (L, D), f32, kind="ExternalInput")
    res_d = nc.dram_tensor("res", (L, D), f32, kind="ExternalInput")
    state_d = nc.dram_tensor("state", (1, D), f32, kind="ExternalInput")
    selidx_d = nc.dram_tensor("selidx", (NCH, JW), i32, kind="ExternalInput")
    cp_d = nc.dram_tensor("cp", (NCH, JW), f32, kind="ExternalInput")
    cif_d = nc.dram_tensor("cif", (1, L + 128), f16, kind="ExternalInput")
    out_d = nc.dram_tensor("out", (L + 1, D), f32, kind="ExternalOutput")

    # constants
    kk = np.arange(128)[:, None]
    mm = np.arange(128)[None, :]
    cst_np = np.concatenate([
        (kk <= mm).astype(np.float32),
        (kk < mm).astype(np.float32),
        np.where(kk < mm, 0.0, -1e5).astype(np.float32),
        np.arange(128, dtype=np.float32).reshape(128, 1)], axis=1)
    cst_d = nc.inline_tensor(cst_np, name="cst")
    ones_d = nc.inline_tensor(np.ones((1, 128), np.float16), name="ones")

    wstart = _window_starts(nblk, ranges)

    with tile.TileContext(nc) as tc:
        with tc.tile_pool(name="persist", bufs=1) as pp, \
             tc.tile_pool(name="resp", bufs=18) as resp, \
             tc.tile_pool(name="psum", bufs=1, space="PSUM") as psp, \
             tc.tile_pool(name="psum_scan", bufs=1, space="PSUM") as pss:

            # ---- small loads ----
            selidx_t = pp.tile([NCH, JW], i32)
            nc.sync.dma_start(out=selidx_t[:], in_=selidx_d[:])
            sm_t = pp.tile([128, 195], f32)
            cp_t = sm_t[:, 0:16]
            nc.sync.dma_start(out=cp_t, in_=cp_d[:])
            state_t = pp.tile([1, D], f32)
            nc.sync.dma_start(out=state_t[:], in_=state_d[:])
            cst_t = pp.tile([128, 385], f32)
            nc.sync.dma_start(out=cst_t[:], in_=cst_d[:])
            lti_t = cst_t[:, 0:128]
            lts_t = cst_t[:, 128:256]
            mb_t = cst_t[:, 256:384]
            iota_t = cst_t[:, 384:385]
            cif_t = pp.tile([1, L + 128], f16)
            nc.sync.dma_start(out=cif_t[:], in_=cif_d[:])
            ones_t = pp.tile([1, 128], f16)
            nc.sync.dma_start(out=ones_t[:], in_=ones_d[:])

            # table, row-major partitions: row t -> partition t%128, block t//128
            TB = pp.tile([128, NTB * D], bf16)

            with tc.tile_pool(name="scanp", bufs=1) as sp:
                # ---- gather selected x rows: CH[c, j*D:(j+1)*D] = x[sel[c*16+j]]
                CH = sp.tile([NCH, JW * D], bf16)
                for j in range(JW):
                    nc.gpsimd.indirect_dma_start(
                        out=CH[:, j * D:(j + 1) * D],
                        out_offset=None,
                        in_=x_d[:],
                        in_offset=IndirectOffsetOnAxis(ap=selidx_t[:, j:j + 1],
                                                       axis=0),
                    )

                # ---- decay + cumprod ----
                dec_t = sm_t[:, 16:32]
                nc.vector.tensor_scalar(out=dec_t, in0=cp_t, scalar1=-1.0,
                                        scalar2=1.0, op0=op.mult, op1=op.add)
                z16 = sm_t[:, 32:48]
                nc.vector.memset(z16, 0.0)
                P_t = sm_t[:, 48:64]
                nc.vector.tensor_tensor_scan(out=P_t, data0=dec_t,
                                             data1=z16, initial=1.0,
                                             op0=op.mult, op1=op.add)

                # ---- cross-chunk carry weights W (only needs cp; emitted early
                # so the scalar engine's Ln/Exp precede its res-DMA stream)
                A_t = sm_t[:, 64:65]
                nc.vector.tensor_scalar(out=A_t, in0=P_t[:, JW - 1:JW],
                                        scalar1=1e-38, scalar2=None, op0=op.max)
                Lc_t = sm_t[:, 65:66]
                nc.scalar.activation(out=Lc_t, in_=A_t, func=AF.Ln)
                nc.vector.tensor_scalar(out=Lc_t, in0=Lc_t, scalar1=-87.0,
                                        scalar2=None, op0=op.max)
                S_ps = pss.tile([128, 1], f32, name="S_ps", tag="small_ps", bufs=2)
                nc.tensor.matmul(out=S_ps[:], lhsT=lti_t, rhs=Lc_t,
                                 start=True, stop=True)
                S_t = sm_t[:, 66:67]
                nc.vector.tensor_copy(out=S_t, in_=S_ps[:])
                S2b_ps = pss.tile([128, 128], f32, name="S2b_ps", tag="small_ps", bufs=2)
                nc.tensor.matmul(out=S2b_ps[:],
                                 lhsT=Lc_t.to_broadcast([128, 128]),
                                 rhs=lts_t, start=True, stop=True)
                W_t = sm_t[:, 67:195]  # f32 workspace for subtract/min
                nc.vector.tensor_scalar(out=W_t, in0=S2b_ps[:],
                                        scalar1=S_t, scalar2=None,
                                        op0=op.subtract)
                nc.vector.tensor_tensor(out=W_t, in0=W_t, in1=mb_t,
                                        op=op.min)
                W_bf = pp.tile([128, 128], bf16)
                nc.scalar.activation(out=W_bf[:], in_=W_t, func=AF.Exp)

                # ---- prefetch residual tiles (scalar HWDGE queue) ----
                res_tiles = []
                for t in range(NLT):
                    r = resp.tile([128, D], f32, name="r", tag="r")
                    nc.scalar.dma_start(out=r[:],
                                        in_=res_d[t * 128:(t + 1) * 128, :])
                    res_tiles.append(r)

                # ---- b scale (in place): CH_j = cp_j * CH_j ----
                for j in range(JW):
                    nc.vector.tensor_scalar(out=CH[:, j * D:(j + 1) * D],
                                            in0=CH[:, j * D:(j + 1) * D],
                                            scalar1=cp_t[:, j:j + 1],
                                            scalar2=None, op0=op.mult)
                # fold initial state into b[0] (chunk 0): b0 += dec0 * state
                nc.vector.scalar_tensor_tensor(out=CH[0:1, 0:D], in0=state_t[:],
                                               scalar=dec_t[0:1, 0:1],
                                               in1=CH[0:1, 0:D],
                                               op0=op.mult, op1=op.add)

                # ---- pass1: in-chunk scan, in place ----
                for j in range(1, JW):
                    nc.vector.scalar_tensor_tensor(
                        out=CH[:, j * D:(j + 1) * D],
                        in0=CH[:, (j - 1) * D:j * D],
                        scalar=dec_t[:, j:j + 1],
                        in1=CH[:, j * D:(j + 1) * D],
                        op0=op.mult, op1=op.add)

                # ---- pass2: carry[c] = sum_{c'<c} W[c',c] * Hend[c'] ----
                carry_ps = pss.tile([128, D], f32, name="carry_ps",
                                    tag="carry_ps")
                hend = CH[:, (JW - 1) * D:JW * D]
                for h in range(2):
                    nc.tensor.matmul(out=carry_ps[:, h * 512:(h + 1) * 512],
                                     lhsT=W_bf[:],
                                     rhs=hend[:, h * 512:(h + 1) * 512],
                                     start=True, stop=True)

                # ---- pass3: E = H + P_j * carry -> bf16 ----
                E_bf = sp.tile([NCH, JW * D], bf16)
                for j in range(JW):
                    nc.vector.scalar_tensor_tensor(
                        out=E_bf[:, j * D:(j + 1) * D],
                        in0=carry_ps[:],
                        scalar=P_t[:, j:j + 1],
                        in1=CH[:, j * D:(j + 1) * D],
                        op0=op.mult, op1=op.add)

                # ---- rearrange (c,j) chunk layout -> row-major blocks ----
                # TB[p, w*D + d] = E(t = 128w + p) = E_bf[8w + p//16, (p%16)*D + d]
                for w in range(NTB):
                    dma_eng = nc.gpsimd if w % 2 == 0 else nc.sync
                    dma_eng.dma_start(
                        out=TB[:, w * D:(w + 1) * D],
                        in_=E_bf[8 * w:8 * w + 8, :])

            # ---- expansion: out_tile = R^T @ table_window + residual ----
            rtp_cm = tc.tile_pool(name="rtp", bufs=4)
            rtp = rtp_cm.__enter__()
            outp_cm = tc.tile_pool(name="outp", bufs=4)
            outp = outp_cm.__enter__()
            for k in range(NLT + 1):
                # ci values for this tile broadcast down all 128 partitions
                cb_ps = pss.tile([128, 128], f32, name="cb_ps", tag="small_ps",
                                 bufs=2)
                nc.tensor.matmul(out=cb_ps[:], lhsT=ones_t[:],
                                 rhs=cif_t[0:1, k * 128:(k + 1) * 128],
                                 start=True, stop=True)
                exp_ps = psp.tile([128, D], f32, name="exp_ps", tag="exp_ps",
                                  bufs=2)
                for bi in range(nblk):
                    w = wstart[k] + bi
                    # R^T[i, l] = (ci[l] - i == 128*w)
                    rt = rtp.tile([128, 128], bf16, name="rt", tag="rt",
                                  bufs=4)
                    nc.vector.tensor_scalar(out=rt[:], in0=cb_ps[:],
                                            scalar1=iota_t,
                                            scalar2=float(128 * w),
                                            op0=op.subtract, op1=op.is_equal)
                    for h in range(2):
                        nc.tensor.matmul(
                            out=exp_ps[:, h * 512:(h + 1) * 512],
                            lhsT=rt[:],
                            rhs=TB[:, w * D + h * 512:w * D + (h + 1) * 512],
                            start=(bi == 0), stop=(bi == nblk - 1))
                if k < NLT:
                    o = outp.tile([128, D], f32, name="o", tag="o")
                    nc.vector.tensor_tensor(out=o[:], in0=res_tiles[k][:],
                                            in1=exp_ps[:], op=op.add)
                    nc.sync.dma_start(out=out_d[k * 128:(k + 1) * 128, :],
                                      in_=o[:])
                else:
                    # tile NLT: every partition holds table[last_idx];
                    # row 0 is new_state
                    ns = outp.tile([1, D], f32, name="ns", tag="ns")
                    nc.vector.tensor_copy(out=ns[:], in_=exp_ps[0:1, :])
                    nc.sync.dma_start(out=out_d[L:L + 1, :], in_=ns[:])
            outp_cm.__exit__(None, None, None)
            rtp_cm.__exit__(None, None, None)

    nc.compile()
    return nc


def _host_prep(prob_row, mask_row):
    """Per-row index/prob prep. Returns (aux input dict, counts, max window excess)."""
    mask = mask_row.astype(bool)
    counts = int(mask.sum())
    sel = np.argsort(~mask, kind="stable")[:M].astype(np.int32)
    valid = (np.arange(M) < counts)
    cp = (prob_row[sel] * valid).astype(np.float32)
    chunk_idx = np.cumsum(mask.astype(np.int64)) - 1
    ci = np.clip(chunk_idx, 0, M - 1)
    last_idx = (counts - 1) if counts > 0 else 0
    cif = np.concatenate([
        np.where(chunk_idx >= 0, ci, M),
        np.full(128, last_idx, np.int64)]).astype(np.float16)
    return {
        "selidx": np.ascontiguousarray(sel.reshape(NCH, JW)),
        "cp": np.ascontiguousarray(cp.reshape(NCH, JW)),
        "cif": np.ascontiguousarray(cif.reshape(1, L + 128)),
    }, counts, ci, last_idx


def _needed_nblk(ci_list, last_list):
    """Smallest nblk whose data-placed static windows cover every tile."""
    ranges = _tile_ranges(ci_list, last_list)
    for nblk in range(2, NTB + 1):
        ws = _window_starts(nblk, ranges)
        ok = True
        for k in range(NLT + 1):
            if ranges[k] is None:
                continue
            lo, hi = ranges[k]
            if lo < 128 * ws[k] or hi >= 128 * (ws[k] + nblk):
                ok = False
                break
        if ok:
            return nblk, ranges
    return NTB, ranges


def kernel(x, residual, prob, token_mask, state):
    from concourse import bass_utils

    auxs, countss, cis, lasts = [], [], [], []
    for b in range(B):
        aux, counts, ci, last_idx = _host_prep(np.asarray(prob[b]),
                                               np.asarray(token_mask[b]))
        auxs.append(aux)
        countss.append(counts)
        lasts.append(last_idx)
        cis.append(np.where(np.asarray(token_mask[b]).astype(bool).cumsum() > 0,
                            ci, -1))
    nblk, ranges = _needed_nblk(cis, lasts)

    key = (nblk, tuple(r if r is None else tuple(r) for r in ranges))
    if key not in _CACHE:
        _CACHE[key] = _build(nblk, ranges)
    nc = _CACHE[key]

    in_maps = []
    for b in range(B):
        in_maps.append({
            "x": np.ascontiguousarray(x[b], dtype=np.float32),
            "res": np.ascontiguousarray(residual[b], dtype=np.float32),
            "state": np.ascontiguousarray(state[b],
                                          dtype=np.float32).reshape(1, D),
            **auxs[b],
        })

    res = bass_utils.run_bass_kernel_spmd(nc, in_maps, core_ids=list(range(B)))
    output = np.stack([res.results[b]["out"][:L] for b in range(B)])
    new_state = np.stack([res.results[b]["out"][L] for b in range(B)])
    for b in range(B):
        if countss[b] == 0:
            # no selected tokens: new_state passes the input state through
            new_state[b] = np.asarray(state[b])
    return output, new_state
